# revision 24
# baseline (speedup 1.0000x reference)
"""Llama RoPE attention (B=2, S=2048, H=2048, 16 heads) on 8 NeuronCores.

Tensor-parallel over heads: core m owns heads {2m, 2m+1}. All matmul inputs
are 16-bit (bf16 activations/weights/q/k, fp16 v and exp tiles); PSUM
accumulation stays fp32.

Schedule (single TileContext, PE-dense, three segments):
  A: batch-0 projections (q/k with RoPE, v) for both heads.
  B: batch-1 q/k projections interleaved 1:1 with batch-0 attention so the
     scalar-engine exp hides under PE work.
  C: batch-1 v projections (x re-DMAed) interleaved with batch-1 attention.

Attention per (batch, head) runs in two query-halves of 1024 so the PV
accumulators need only 2 PSUM banks.  Softmax denominator is computed with
a ones-matmul partition reduce on the PE (replicated across partitions),
inverted with a fast approximate reciprocal on DVE, and multiplied into the
PV result -- no GpSimd reduce, no DMA round-trip.

PSUM budget (8 banks): proj ring 2 + score/denominator ring 4 + PV 2.

Output is the transposed flattened attention output [256, 4096] per core;
the host stacks core outputs and transposes back.
"""

import math
import os
import sys

for _p in ("/opt/trn_rl_repo", "/root/.axon_site/_ro/trn_rl_repo"):
    if os.path.isdir(_p) and _p not in sys.path:
        sys.path.insert(0, _p)
        break

import numpy as np
import ml_dtypes

import concourse.bass as bass
import concourse.bacc as bacc
import concourse.mybir as mybir
from concourse import bass_isa, tile
from concourse.bass_utils import run_bass_kernel_spmd

N_CORES = 8
HIDDEN = 2048
N_HEAD = 16
HEAD_DIM = 128
B = 2
S = 2048
NTOK = B * S  # 4096
OPC = 256  # output cols per core = 2 heads * 128
KI = HIDDEN // 128  # 16 contraction tiles
NBLK = 512  # token block for projection groups
NB = NTOK // NBLK  # 8
SCALE = 1.0 / math.sqrt(HEAD_DIM)
F32 = mybir.dt.float32
F16 = mybir.dt.float16
BF16 = mybir.dt.bfloat16
EXP = mybir.ActivationFunctionType.Exp

_CACHE = {}

# test.py can read this after calling kernel() with BASS_TRACE=1
LAST_RESULT = None


def _roundrobin(*gens):
    gens = [iter(g) for g in gens]
    while gens:
        for g in list(gens):
            try:
                next(g)
            except StopIteration:
                gens.remove(g)


def _build_nc():
    nc = bacc.Bacc("TRN2", target_bir_lowering=False, debug=False,
                   num_devices=N_CORES)
    # packed: xT[p, ((nb*4+c)*2048 + t*512 + n)] = x.T[c*512+t*128+p, nb*512+n]
    xT = nc.dram_tensor("xT", [128, NB * 4 * 2048], BF16,
                        kind="ExternalInput")
    # packed: w*T[p, t*256+o] = w[o_local, t*128+p]
    wqT = nc.dram_tensor("wqT", [128, KI * 256], BF16, kind="ExternalInput")
    wkT = nc.dram_tensor("wkT", [128, KI * 256], BF16, kind="ExternalInput")
    wvT = nc.dram_tensor("wvT", [128, KI * 256], BF16, kind="ExternalInput")
    cosT = nc.dram_tensor("cosT", [HEAD_DIM, S], F32, kind="ExternalInput")
    sinS = nc.dram_tensor("sinS", [HEAD_DIM, S], F32, kind="ExternalInput")
    outT = nc.dram_tensor("outT", [OPC, NTOK], F32, kind="ExternalOutput")

    with tile.TileContext(nc) as tc:
        with (
            tc.tile_pool(name="const", bufs=1) as cp,
            tc.tile_pool(name="qk_res", bufs=1) as qkp,
            tc.tile_pool(name="v_res", bufs=1) as vp,
            tc.tile_pool(name="w", bufs=1) as wp,
            tc.tile_pool(name="x", bufs=16) as xp,
            tc.tile_pool(name="rope_tmp", bufs=2) as rtp,
            tc.tile_pool(name="e", bufs=3) as ep,
            tc.tile_pool(name="acc", bufs=2) as accp,
            tc.tile_pool(name="epi", bufs=2) as epp,
            tc.tile_pool(name="osb", bufs=3) as osp,
            tc.tile_pool(name="ps_p", bufs=2, space="PSUM") as ppp,
            tc.tile_pool(name="ps_s", bufs=2, space="PSUM") as psp,
            tc.tile_pool(name="ps_o", bufs=1, space="PSUM") as pop,
        ):
            # ---- resident tiles ----
            q_sb = [qkp.tile([128, NTOK], BF16, tag=f"q{o}", name=f"q_sb{o}")
                    for o in range(2)]
            k_sb = [qkp.tile([128, NTOK], BF16, tag=f"k{o}", name=f"k_sb{o}")
                    for o in range(2)]
            # [n%128, (jg d)] fp16; jg = 512-token block * 4 + j
            v_sb = vp.tile([128, 32 * 256], F16, tag="v")
            cos_sb = cp.tile([128, S], F32, tag="cos")
            sin_sb = cp.tile([128, S], F32, tag="sin")
            ones_sb = cp.tile([128, 128], F16, tag="ones")
            nc.gpsimd.memset(ones_sb[:], 1.0)

            w_sb = {nm: wp.tile([128, KI * 256], BF16, tag=f"w{nm}",
                                name=f"w_{nm}") for nm in ("q", "k", "v")}

            def load_w_chunk(nm, drt, t0, t1):
                nc.sync.dma_start(w_sb[nm][:, t0 * 256:t1 * 256],
                                  drt[:, t0 * 256:t1 * 256])

            x_tiles = {}  # nb -> [4 tiles]

            def load_x(nb):
                n0 = nb * NBLK
                xc = []
                for c in range(4):
                    xt = xp.tile([128, 4 * NBLK], BF16, tag="x", name="xt")
                    nc.sync.dma_start(
                        xt[:],
                        xT[:, (nb * 4 + c) * 2048:(nb * 4 + c + 1) * 2048])
                    xc.append(xt)
                x_tiles[nb] = xc
                return xc

            # ---- startup DMAs: matmul-critical bytes first ----
            def load_x_tile(nb, c):
                xt = xp.tile([128, 4 * NBLK], BF16, tag="x", name="xt")
                nc.sync.dma_start(
                    xt[:],
                    xT[:, (nb * 4 + c) * 2048:(nb * 4 + c + 1) * 2048])
                return xt

            cs_loaded = set()

            def load_cs(s0):
                if s0 in cs_loaded:
                    return
                cs_loaded.add(s0)
                nc.sync.dma_start(cos_sb[:, s0:s0 + 512], cosT[:, s0:s0 + 512])
                nc.sync.dma_start(sin_sb[:, s0:s0 + 512], sinS[:, s0:s0 + 512])

            load_w_chunk("q", wqT, 0, 4)
            xc00 = load_x_tile(0, 0)
            load_w_chunk("q", wqT, 4, 8)
            xc01 = load_x_tile(0, 1)
            load_w_chunk("q", wqT, 8, KI)
            xc02 = load_x_tile(0, 2)
            xc03 = load_x_tile(0, 3)
            x_tiles[0] = [xc00, xc01, xc02, xc03]
            load_cs(0)
            load_w_chunk("k", wkT, 0, 4)
            load_w_chunk("k", wkT, 4, KI)
            load_w_chunk("v", wvT, 0, KI)

            # ---- projection generator ----
            # kinds: "qk" emits q/k groups (+RoPE), "v" emits v groups,
            # "qkv" both.  Yields after every 2 matmuls (one PE chunk).
            def proj_gen(nbs, kinds, vcopy_eng, del_x=True):
                for ni, nb in enumerate(nbs):
                    if nb not in x_tiles:
                        load_x(nb)
                    nxt = nbs[ni + 1] if ni + 1 < len(nbs) else None
                    if nxt is not None and nxt not in x_tiles:
                        load_x(nxt)
                    if nxt is not None and kinds in ("qk", "qkv"):
                        load_cs((nxt % 4) * NBLK)
                    xc = x_tiles[nb]
                    n0 = nb * NBLK
                    s0 = (nb % 4) * NBLK
                    if kinds in ("qk", "qkv"):
                        for nm, outsb in (("q", q_sb), ("k", k_sb)):
                            for o in range(2):
                                pq = ppp.tile([128, NBLK], F32, tag="pp",
                                              name="pq")
                                for c in range(4):
                                    for t in range(4):
                                        i = c * 4 + t
                                        nc.tensor.matmul(
                                            pq[:],
                                            w_sb[nm][:, i * 256 + o * 128:
                                                     i * 256 + o * 128 + 128],
                                            xc[c][:, t * NBLK:(t + 1) * NBLK],
                                            start=(i == 0),
                                            stop=(i == KI - 1),
                                        )
                                        if i % 2 == 1:
                                            yield
                                # RoPE: dst = pq*cos + rot(pq)*sinS (bf16 out)
                                t1 = rtp.tile([128, NBLK], F32, tag="t1")
                                nc.vector.tensor_mul(
                                    t1[:], pq[:], cos_sb[:, s0:s0 + NBLK])
                                t2 = rtp.tile([128, NBLK], F32, tag="t2")
                                nc.vector.tensor_mul(
                                    t2[0:64, :], pq[64:128, :],
                                    sin_sb[0:64, s0:s0 + NBLK])
                                nc.vector.tensor_mul(
                                    t2[64:128, :], pq[0:64, :],
                                    sin_sb[64:128, s0:s0 + NBLK])
                                nc.vector.tensor_add(
                                    outsb[o][:, n0:n0 + NBLK], t1[:], t2[:])
                    if kinds in ("v", "qkv"):
                        for j in range(4):
                            pv = ppp.tile([128, NBLK], F32, tag="pp",
                                          name="pv")
                            for c in range(4):
                                for t in range(4):
                                    i = c * 4 + t
                                    nc.tensor.matmul(
                                        pv[:, 0:256],
                                        xc[c][:, t * NBLK + j * 128:
                                              t * NBLK + j * 128 + 128],
                                        w_sb["v"][:, i * 256:(i + 1) * 256],
                                        start=(i == 0), stop=(i == KI - 1),
                                    )
                                    if i % 2 == 1:
                                        yield
                            jg = nb * 4 + j
                            dst = v_sb[:, jg * 256:(jg + 1) * 256]
                            if vcopy_eng == "scalar":
                                nc.scalar.copy(dst, pv[:, 0:256])
                            else:
                                nc.vector.tensor_copy(dst, pv[:, 0:256])
                    if del_x:
                        del x_tiles[nb]

            # ---- attention generator: one (batch, head) pair ----
            # Two query-halves of 1024; per half a 16-step sk loop with
            # one-iteration PV lag so exp latency hides under interleaved
            # PE work.  Yields twice per sk step.
            def attn_gen(b, h):
                for half in range(2):
                    q0 = b * 2048 + half * 1024
                    po = [pop.tile([128, NBLK], F32, tag=f"po{j}",
                                   name=f"po{j}") for j in range(2)]
                    acc = accp.tile([128, 1024], F16, tag="acc", name="acc")
                    prev_e = None
                    for sk in range(16):
                        kt = k_sb[h][:, b * 2048 + sk * 128:
                                     b * 2048 + sk * 128 + 128]
                        ps = psp.tile([128, 1024], F32, tag="ps", name="ps")
                        for q in range(2):
                            nc.tensor.matmul(
                                ps[:, q * NBLK:(q + 1) * NBLK],
                                kt,
                                q_sb[h][:, q0 + q * NBLK:
                                        q0 + (q + 1) * NBLK],
                                start=True, stop=True,
                            )
                        e = ep.tile([128, 1024], F16, tag="e", name="e")
                        nc.scalar.activation(e[:], ps[:], EXP, scale=SCALE)
                        if sk == 0:
                            nc.vector.tensor_copy(acc[:], e[:])
                        else:
                            nc.vector.tensor_add(acc[:], acc[:], e[:])
                        yield
                        if prev_e is not None:
                            pjg, pe_t = prev_e
                            vt = v_sb[:, pjg * 256 + h * 128:
                                      pjg * 256 + h * 128 + 128]
                            for j in range(2):
                                nc.tensor.matmul(
                                    po[j][:],
                                    vt,
                                    pe_t[:, j * NBLK:(j + 1) * NBLK],
                                    start=(sk == 1), stop=(sk == 15),
                                )
                        yield
                        prev_e = (b * 16 + sk, e)
                    # last PV
                    pjg, pe_t = prev_e
                    vt = v_sb[:, pjg * 256 + h * 128:pjg * 256 + h * 128 + 128]
                    for j in range(2):
                        nc.tensor.matmul(
                            po[j][:], vt, pe_t[:, j * NBLK:(j + 1) * NBLK],
                            start=False, stop=True,
                        )
                    # epilogue: dn = ones.T @ acc (partition-sum,
                    # replicated); dn lives in the proj PSUM ring so the
                    # next half's score matmuls don't wait on its readers
                    dns = []
                    for j in range(2):
                        dn = ppp.tile([128, NBLK], F32, tag="pp", name="dn")
                        nc.tensor.matmul(
                            dn[:], ones_sb[:],
                            acc[:, j * NBLK:(j + 1) * NBLK],
                            start=True, stop=True,
                        )
                        dns.append(dn)
                    yield
                    rc = epp.tile([128, 1024], F32, tag="rc", name="rc")
                    for j in range(2):
                        nc.vector.reciprocal_approx_fast(
                            out=rc[:, j * NBLK:(j + 1) * NBLK],
                            in_=dns[j][:])
                    for j in range(2):
                        osb = osp.tile([128, NBLK], F32, tag="osb",
                                       name="osb")
                        nc.vector.tensor_mul(
                            osb[:], po[j][:], rc[:, j * NBLK:(j + 1) * NBLK])
                        nc.sync.dma_start(
                            outT[h * 128:(h + 1) * 128,
                                 q0 + j * NBLK:q0 + (j + 1) * NBLK],
                            osb[:])
                    yield

            def chain(*gs):
                for g in gs:
                    yield from g

            # segment A: batch-0 projections (q/k/v)
            for _ in proj_gen([0, 1, 2, 3], "qkv", "scalar"):
                pass
            # segment B: batch-1 q/k projections | batch-0 attention
            # (x tiles for nb 4-7 stay resident for segment C's v groups)
            _roundrobin(chain(attn_gen(0, 0), attn_gen(0, 1)),
                        proj_gen([4, 5, 6, 7], "qk", None, del_x=False))
            # segment C: batch-1 v projections (front-loaded so production
            # stays one 128-token group ahead of PV consumption) | batch-1
            # attention
            vgen = iter(proj_gen([4, 5, 6, 7], "v", "vector"))
            agen = iter(chain(attn_gen(1, 0), attn_gen(1, 1)))
            for _ in range(16):
                next(vgen, None)
            while True:
                try:
                    next(agen)
                except StopIteration:
                    break
                for _ in range(4):
                    next(vgen, None)
            for _ in vgen:
                pass
    nc.compile()
    return nc


def _get_nc():
    if "nc" not in _CACHE:
        _CACHE["nc"] = _build_nc()
    return _CACHE["nc"]


def _cos_sin():
    if "cs" not in _CACHE:
        half = np.arange(0, HEAD_DIM, 2, dtype=np.float32)[: HEAD_DIM // 2]
        freq = (1.0 / 10000.0 ** (half / HEAD_DIM)).astype(np.float32)
        t = np.arange(S, dtype=np.float32)
        freqs = np.outer(t, freq).astype(np.float32)  # [S, 64]
        emb = np.concatenate([freqs, freqs], axis=1)  # [S, 128]
        cosT = np.ascontiguousarray(np.cos(emb).astype(np.float32).T)
        sinT = np.ascontiguousarray(np.sin(emb).astype(np.float32).T)
        sinS = np.concatenate([-sinT[0:64], sinT[64:128]], axis=0)
        _CACHE["cs"] = (cosT, np.ascontiguousarray(sinS))
    return _CACHE["cs"]


def kernel(x, wq, wk, wv):
    global LAST_RESULT
    nc = _get_nc()
    cosT, sinS = _cos_sin()
    bf16 = ml_dtypes.bfloat16
    x2 = np.asarray(x, dtype=np.float32).reshape(NTOK, HIDDEN).T  # [H, NTOK]
    # pack to [p, nb, c, t, n] so each (nb, c) tile DMA is contiguous
    xP = np.ascontiguousarray(
        x2.reshape(4, 4, 128, NB, NBLK).transpose(2, 3, 0, 1, 4)
        .reshape(128, NB * 4 * 2048)).astype(bf16)

    def packw(w, sl):
        wt = np.asarray(w)[sl].T  # [HIDDEN, OPC]
        return np.ascontiguousarray(
            wt.reshape(KI, 128, OPC).transpose(1, 0, 2)
            .reshape(128, KI * OPC)).astype(bf16)

    in_maps = []
    for m in range(N_CORES):
        sl = slice(m * OPC, (m + 1) * OPC)
        in_maps.append({
            "xT": xP,
            "wqT": packw(wq, sl),
            "wkT": packw(wk, sl),
            "wvT": packw(wv, sl),
            "cosT": cosT,
            "sinS": sinS,
        })
    res = run_bass_kernel_spmd(nc, in_maps, core_ids=list(range(N_CORES)))
    LAST_RESULT = res
    big = np.concatenate([r["outT"] for r in res.results], axis=0)
    return np.ascontiguousarray(big.T).reshape(B, S, HIDDEN).astype(np.float32)


if __name__ == "__main__":
    _get_nc()
    print("build OK")


# revision 25
# speedup vs baseline: 1.0116x; 1.0116x over previous
"""Llama RoPE attention (B=2, S=2048, H=2048, 16 heads) on 8 NeuronCores.

Tensor-parallel over heads: core m owns heads {2m, 2m+1}. All matmul inputs
are 16-bit (bf16 activations/weights/q/k, fp16 v and exp tiles); PSUM
accumulation stays fp32.

Schedule (single TileContext, PE-dense, three segments):
  A: batch-0 projections (q/k with RoPE, v) for both heads.
  B: batch-1 q/k projections interleaved 1:1 with batch-0 attention so the
     scalar-engine exp hides under PE work.
  C: batch-1 v projections (x re-DMAed) interleaved with batch-1 attention.

Attention per (batch, head) runs in two query-halves of 1024 so the PV
accumulators need only 2 PSUM banks.  Softmax denominator is computed with
a ones-matmul partition reduce on the PE (replicated across partitions),
inverted with a fast approximate reciprocal on DVE, and multiplied into the
PV result -- no GpSimd reduce, no DMA round-trip.

PSUM budget (8 banks): proj ring 2 + score/denominator ring 4 + PV 2.

Output is the transposed flattened attention output [256, 4096] per core;
the host stacks core outputs and transposes back.
"""

import math
import os
import sys

for _p in ("/opt/trn_rl_repo", "/root/.axon_site/_ro/trn_rl_repo"):
    if os.path.isdir(_p) and _p not in sys.path:
        sys.path.insert(0, _p)
        break

import numpy as np
import ml_dtypes

import concourse.bass as bass
import concourse.bacc as bacc
import concourse.mybir as mybir
from concourse import bass_isa, tile
from concourse.bass_utils import run_bass_kernel_spmd

N_CORES = 8
HIDDEN = 2048
N_HEAD = 16
HEAD_DIM = 128
B = 2
S = 2048
NTOK = B * S  # 4096
OPC = 256  # output cols per core = 2 heads * 128
KI = HIDDEN // 128  # 16 contraction tiles
NBLK = 512  # token block for projection groups
NB = NTOK // NBLK  # 8
SCALE = 1.0 / math.sqrt(HEAD_DIM)
F32 = mybir.dt.float32
F16 = mybir.dt.float16
BF16 = mybir.dt.bfloat16
EXP = mybir.ActivationFunctionType.Exp

_CACHE = {}

# test.py can read this after calling kernel() with BASS_TRACE=1
LAST_RESULT = None


def _roundrobin(*gens):
    gens = [iter(g) for g in gens]
    while gens:
        for g in list(gens):
            try:
                next(g)
            except StopIteration:
                gens.remove(g)


def _build_nc():
    nc = bacc.Bacc("TRN2", target_bir_lowering=False, debug=False,
                   num_devices=N_CORES)
    # packed: xT[p, ((nb*4+c)*2048 + t*512 + n)] = x.T[c*512+t*128+p, nb*512+n]
    xT = nc.dram_tensor("xT", [128, NB * 4 * 2048], BF16,
                        kind="ExternalInput")
    # packed: w*T[p, t*256+o] = w[o_local, t*128+p]
    wqT = nc.dram_tensor("wqT", [128, KI * 256], BF16, kind="ExternalInput")
    wkT = nc.dram_tensor("wkT", [128, KI * 256], BF16, kind="ExternalInput")
    wvT = nc.dram_tensor("wvT", [128, KI * 256], BF16, kind="ExternalInput")
    cosT = nc.dram_tensor("cosT", [HEAD_DIM, S], F32, kind="ExternalInput")
    sinS = nc.dram_tensor("sinS", [HEAD_DIM, S], F32, kind="ExternalInput")
    outT = nc.dram_tensor("outT", [OPC, NTOK], F32, kind="ExternalOutput")

    with tile.TileContext(nc) as tc:
        with (
            tc.tile_pool(name="const", bufs=1) as cp,
            tc.tile_pool(name="qk_res", bufs=1) as qkp,
            tc.tile_pool(name="v_res", bufs=1) as vp,
            tc.tile_pool(name="w", bufs=1) as wp,
            tc.tile_pool(name="x", bufs=16) as xp,
            tc.tile_pool(name="rope_tmp", bufs=2) as rtp,
            tc.tile_pool(name="e", bufs=3) as ep,
            tc.tile_pool(name="acc", bufs=2) as accp,
            tc.tile_pool(name="epi", bufs=2) as epp,
            tc.tile_pool(name="osb", bufs=3) as osp,
            tc.tile_pool(name="ps_p", bufs=2, space="PSUM") as ppp,
            tc.tile_pool(name="ps_s", bufs=2, space="PSUM") as psp,
            tc.tile_pool(name="ps_o", bufs=1, space="PSUM") as pop,
        ):
            # ---- resident tiles ----
            q_sb = [qkp.tile([128, NTOK], BF16, tag=f"q{o}", name=f"q_sb{o}")
                    for o in range(2)]
            k_sb = [qkp.tile([128, NTOK], BF16, tag=f"k{o}", name=f"k_sb{o}")
                    for o in range(2)]
            # [n%128, (jg d)] fp16; jg = 512-token block * 4 + j
            v_sb = vp.tile([128, 32 * 256], F16, tag="v")
            cos_sb = cp.tile([128, S], F32, tag="cos")
            sin_sb = cp.tile([128, S], F32, tag="sin")
            ones_sb = cp.tile([128, 128], F16, tag="ones")
            nc.gpsimd.memset(ones_sb[:], 1.0)

            w_sb = {nm: wp.tile([128, KI * 256], BF16, tag=f"w{nm}",
                                name=f"w_{nm}") for nm in ("q", "k", "v")}

            def load_w_chunk(nm, drt, t0, t1):
                nc.sync.dma_start(w_sb[nm][:, t0 * 256:t1 * 256],
                                  drt[:, t0 * 256:t1 * 256])

            x_tiles = {}  # nb -> [4 tiles]

            def load_x(nb):
                n0 = nb * NBLK
                xc = []
                for c in range(4):
                    xt = xp.tile([128, 4 * NBLK], BF16, tag="x", name="xt")
                    nc.sync.dma_start(
                        xt[:],
                        xT[:, (nb * 4 + c) * 2048:(nb * 4 + c + 1) * 2048])
                    xc.append(xt)
                x_tiles[nb] = xc
                return xc

            # ---- startup DMAs: matmul-critical bytes first ----
            def load_x_tile(nb, c):
                xt = xp.tile([128, 4 * NBLK], BF16, tag="x", name="xt")
                nc.sync.dma_start(
                    xt[:],
                    xT[:, (nb * 4 + c) * 2048:(nb * 4 + c + 1) * 2048])
                return xt

            cs_loaded = set()

            def load_cs(s0):
                if s0 in cs_loaded:
                    return
                cs_loaded.add(s0)
                nc.sync.dma_start(cos_sb[:, s0:s0 + 512], cosT[:, s0:s0 + 512])
                nc.sync.dma_start(sin_sb[:, s0:s0 + 512], sinS[:, s0:s0 + 512])

            load_w_chunk("q", wqT, 0, 4)
            xc00 = load_x_tile(0, 0)
            load_w_chunk("q", wqT, 4, 8)
            xc01 = load_x_tile(0, 1)
            load_w_chunk("q", wqT, 8, KI)
            xc02 = load_x_tile(0, 2)
            xc03 = load_x_tile(0, 3)
            x_tiles[0] = [xc00, xc01, xc02, xc03]
            load_cs(0)
            load_w_chunk("k", wkT, 0, 4)
            load_w_chunk("k", wkT, 4, KI)
            load_w_chunk("v", wvT, 0, KI)

            # ---- projection generator ----
            # kinds: "qk" emits q/k groups (+RoPE), "v" emits v groups,
            # "qkv" both.  Yields after every 2 matmuls (one PE chunk).
            def proj_gen(nbs, kinds, vcopy_eng, del_x=True):
                for ni, nb in enumerate(nbs):
                    if nb not in x_tiles:
                        load_x(nb)
                    nxt = nbs[ni + 1] if ni + 1 < len(nbs) else None
                    if nxt is not None and nxt not in x_tiles:
                        load_x(nxt)
                    if nxt is not None and kinds in ("qk", "qkv"):
                        load_cs((nxt % 4) * NBLK)
                    xc = x_tiles[nb]
                    n0 = nb * NBLK
                    s0 = (nb % 4) * NBLK
                    if kinds in ("qk", "qkv"):
                        for nm, outsb in (("q", q_sb), ("k", k_sb)):
                            for o in range(2):
                                pq = ppp.tile([128, NBLK], F32, tag="pp",
                                              name="pq")
                                for c in range(4):
                                    for t in range(4):
                                        i = c * 4 + t
                                        nc.tensor.matmul(
                                            pq[:],
                                            w_sb[nm][:, i * 256 + o * 128:
                                                     i * 256 + o * 128 + 128],
                                            xc[c][:, t * NBLK:(t + 1) * NBLK],
                                            start=(i == 0),
                                            stop=(i == KI - 1),
                                        )
                                        if i % 2 == 1:
                                            yield
                                # RoPE: dst = pq*cos + rot(pq)*sinS (bf16 out)
                                t1 = rtp.tile([128, NBLK], F32, tag="t1")
                                nc.vector.tensor_mul(
                                    t1[:], pq[:], cos_sb[:, s0:s0 + NBLK])
                                t2 = rtp.tile([128, NBLK], F32, tag="t2")
                                nc.vector.tensor_mul(
                                    t2[0:64, :], pq[64:128, :],
                                    sin_sb[0:64, s0:s0 + NBLK])
                                nc.vector.tensor_mul(
                                    t2[64:128, :], pq[0:64, :],
                                    sin_sb[64:128, s0:s0 + NBLK])
                                nc.vector.tensor_add(
                                    outsb[o][:, n0:n0 + NBLK], t1[:], t2[:])
                    if kinds in ("v", "qkv"):
                        for j in range(4):
                            pv = ppp.tile([128, NBLK], F32, tag="pp",
                                          name="pv")
                            for c in range(4):
                                for t in range(4):
                                    i = c * 4 + t
                                    nc.tensor.matmul(
                                        pv[:, 0:256],
                                        xc[c][:, t * NBLK + j * 128:
                                              t * NBLK + j * 128 + 128],
                                        w_sb["v"][:, i * 256:(i + 1) * 256],
                                        start=(i == 0), stop=(i == KI - 1),
                                    )
                                    if i % 2 == 1:
                                        yield
                            jg = nb * 4 + j
                            dst = v_sb[:, jg * 256:(jg + 1) * 256]
                            if vcopy_eng == "scalar":
                                nc.scalar.copy(dst, pv[:, 0:256])
                            else:
                                nc.vector.tensor_copy(dst, pv[:, 0:256])
                    if del_x:
                        del x_tiles[nb]

            # ---- attention generator: one (batch, head) pair ----
            # Two query-halves of 1024; per half a 16-step sk loop with
            # one-iteration PV lag so exp latency hides under interleaved
            # PE work.  Yields twice per sk step.
            def attn_gen(b, h):
                for half in range(2):
                    q0 = b * 2048 + half * 1024
                    po = [pop.tile([128, NBLK], F32, tag=f"po{j}",
                                   name=f"po{j}") for j in range(2)]
                    acc = accp.tile([128, 1024], F16, tag="acc", name="acc")
                    prev_e = None
                    for sk in range(16):
                        kt = k_sb[h][:, b * 2048 + sk * 128:
                                     b * 2048 + sk * 128 + 128]
                        ps = psp.tile([128, 1024], F32, tag="ps", name="ps")
                        for q in range(2):
                            nc.tensor.matmul(
                                ps[:, q * NBLK:(q + 1) * NBLK],
                                kt,
                                q_sb[h][:, q0 + q * NBLK:
                                        q0 + (q + 1) * NBLK],
                                start=True, stop=True,
                            )
                        e = ep.tile([128, 1024], F16, tag="e", name="e")
                        nc.scalar.activation(e[:], ps[:], EXP, scale=SCALE)
                        if sk == 0:
                            nc.vector.tensor_copy(acc[:], e[:])
                        else:
                            nc.vector.tensor_add(acc[:], acc[:], e[:])
                        yield
                        if prev_e is not None:
                            pjg, pe_t = prev_e
                            vt = v_sb[:, pjg * 256 + h * 128:
                                      pjg * 256 + h * 128 + 128]
                            for j in range(2):
                                nc.tensor.matmul(
                                    po[j][:],
                                    vt,
                                    pe_t[:, j * NBLK:(j + 1) * NBLK],
                                    start=(sk == 1), stop=(sk == 15),
                                )
                        yield
                        prev_e = (b * 16 + sk, e)
                    # last PV
                    pjg, pe_t = prev_e
                    vt = v_sb[:, pjg * 256 + h * 128:pjg * 256 + h * 128 + 128]
                    for j in range(2):
                        nc.tensor.matmul(
                            po[j][:], vt, pe_t[:, j * NBLK:(j + 1) * NBLK],
                            start=False, stop=True,
                        )
                    # epilogue: dn = ones.T @ acc (partition-sum, replicated)
                    dn = psp.tile([128, 1024], F32, tag="ps", name="dn")
                    for j in range(2):
                        nc.tensor.matmul(
                            dn[:, j * NBLK:(j + 1) * NBLK],
                            ones_sb[:],
                            acc[:, j * NBLK:(j + 1) * NBLK],
                            start=True, stop=True,
                        )
                    yield
                    rc = epp.tile([128, 1024], F32, tag="rc", name="rc")
                    nc.vector.reciprocal_approx_fast(out=rc[:], in_=dn[:])
                    for j in range(2):
                        osb = osp.tile([128, NBLK], F32, tag="osb",
                                       name="osb")
                        nc.vector.tensor_mul(
                            osb[:], po[j][:], rc[:, j * NBLK:(j + 1) * NBLK])
                        nc.sync.dma_start(
                            outT[h * 128:(h + 1) * 128,
                                 q0 + j * NBLK:q0 + (j + 1) * NBLK],
                            osb[:])
                    yield

            def chain(*gs):
                for g in gs:
                    yield from g

            # segment A: batch-0 projections (q/k/v)
            for _ in proj_gen([0, 1, 2, 3], "qkv", "scalar"):
                pass
            # segment B: batch-1 q/k projections | batch-0 attention
            # (x tiles for nb 4-7 stay resident for segment C's v groups)
            _roundrobin(chain(attn_gen(0, 0), attn_gen(0, 1)),
                        proj_gen([4, 5, 6, 7], "qk", None, del_x=False))
            # segment C: batch-1 v projections (front-loaded so production
            # stays one 128-token group ahead of PV consumption) | batch-1
            # attention
            vgen = iter(proj_gen([4, 5, 6, 7], "v", "vector"))
            agen = iter(chain(attn_gen(1, 0), attn_gen(1, 1)))
            for _ in range(16):
                next(vgen, None)
            while True:
                try:
                    next(agen)
                except StopIteration:
                    break
                for _ in range(4):
                    next(vgen, None)
            for _ in vgen:
                pass
    nc.compile()
    return nc


def _get_nc():
    if "nc" not in _CACHE:
        _CACHE["nc"] = _build_nc()
    return _CACHE["nc"]


def _cos_sin():
    if "cs" not in _CACHE:
        half = np.arange(0, HEAD_DIM, 2, dtype=np.float32)[: HEAD_DIM // 2]
        freq = (1.0 / 10000.0 ** (half / HEAD_DIM)).astype(np.float32)
        t = np.arange(S, dtype=np.float32)
        freqs = np.outer(t, freq).astype(np.float32)  # [S, 64]
        emb = np.concatenate([freqs, freqs], axis=1)  # [S, 128]
        cosT = np.ascontiguousarray(np.cos(emb).astype(np.float32).T)
        sinT = np.ascontiguousarray(np.sin(emb).astype(np.float32).T)
        sinS = np.concatenate([-sinT[0:64], sinT[64:128]], axis=0)
        _CACHE["cs"] = (cosT, np.ascontiguousarray(sinS))
    return _CACHE["cs"]


def kernel(x, wq, wk, wv):
    global LAST_RESULT
    nc = _get_nc()
    cosT, sinS = _cos_sin()
    bf16 = ml_dtypes.bfloat16
    x2 = np.asarray(x, dtype=np.float32).reshape(NTOK, HIDDEN).T  # [H, NTOK]
    # pack to [p, nb, c, t, n] so each (nb, c) tile DMA is contiguous
    xP = np.ascontiguousarray(
        x2.reshape(4, 4, 128, NB, NBLK).transpose(2, 3, 0, 1, 4)
        .reshape(128, NB * 4 * 2048)).astype(bf16)

    def packw(w, sl):
        wt = np.asarray(w)[sl].T  # [HIDDEN, OPC]
        return np.ascontiguousarray(
            wt.reshape(KI, 128, OPC).transpose(1, 0, 2)
            .reshape(128, KI * OPC)).astype(bf16)

    in_maps = []
    for m in range(N_CORES):
        sl = slice(m * OPC, (m + 1) * OPC)
        in_maps.append({
            "xT": xP,
            "wqT": packw(wq, sl),
            "wkT": packw(wk, sl),
            "wvT": packw(wv, sl),
            "cosT": cosT,
            "sinS": sinS,
        })
    res = run_bass_kernel_spmd(nc, in_maps, core_ids=list(range(N_CORES)))
    LAST_RESULT = res
    big = np.concatenate([r["outT"] for r in res.results], axis=0)
    return np.ascontiguousarray(big.T).reshape(B, S, HIDDEN).astype(np.float32)


if __name__ == "__main__":
    _get_nc()
    print("build OK")
